# revision 69
# baseline (speedup 1.0000x reference)
"""Trainium2 Bass kernel for nn_DIOU3DLoss (mmcv diff_iou_rotated_3d style).

Self-contained: hardcodes shapes/sharding. kernel(pred, target) takes FULL
inputs [262144, 7] f32, shards the box axis across 8 NeuronCores, runs one
SPMD Bass program, and reduces the per-core partial sums to the scalar mean
loss on the host (the unshard step).

v2: the 2D polygon-clipping machinery runs in fp16 (DVE 2x/4x perf modes),
the sort pivot is o/2 (box2 center in frame1, halved) instead of the vertex
centroid (same cyclic order around any interior point of a convex polygon),
three of the four last-valid scans run on GpSimd, reductions are short
tree-sums, and fp16 range is protected by clamping the edge-parameter and
the phantom-edge reciprocals. The z/vol/ctd/cnd/did tail stays f32 on
Pool/Act as before.
"""

import numpy as np

import concourse.bass as bass
import concourse.tile as tile
from concourse import mybir
from concourse.bass_utils import run_bass_kernel_spmd

P = 128
NCORES = 8
PI = float(np.pi)
TOL = 1e-6
LOSS_EPS = 1e-6
F32 = mybir.dt.float32
F16 = mybir.dt.float16
I16 = mybir.dt.int16
Alu = mybir.AluOpType
Act = mybir.ActivationFunctionType
AxX = mybir.AxisListType.X
RCLAMP = 1024.0  # keep 1/D (and sj8 = num*1/D, |num|<=8) fp16-finite


def _ap(t, off, dims):
    base = t[:, :]
    return bass.AP(base.tensor, base.offset + off, [base.ap[0]] + dims)


def bc_pair(t):
    base = t[:, :]
    return bass.AP(base.tensor, base.offset, [base.ap[0], [0, 2],
                                              [1, base.ap[-1][1]]])


def _legalize_sync(nc):
    """Split multi-wait instructions: this walrus build encodes at most one
    sem-wait (+ one update) per instruction, but Tile's scheduler emits
    several. Carry the extra waits on preceding same-engine NoOps."""
    k = 0
    for fn in nc.m.functions:
        for bl in fn.blocks:
            il = bl.instructions
            new = []
            for inst in il:
                si = getattr(inst, "sync_info", None)
                if si is not None and si.on_wait and len(si.on_wait) > 1:
                    waits = list(si.on_wait)
                    for w in waits[:-1]:
                        k += 1
                        nop = mybir.InstNoOp(name=f"WSPLIT-{k}", ins=[],
                                             outs=[])
                        nop.engine = inst.engine
                        nop.sync_info = mybir.SyncInfo(on_wait=[w],
                                                       on_update=[])
                        new.append(nop)
                    inst.sync_info = mybir.SyncInfo(
                        on_wait=[waits[-1]],
                        on_update=list(si.on_update or []))
                new.append(inst)
            il[:] = new


def build_nc(F, legalize=True):
    """Bass program for one core's shard of P*F boxes.

    DRAM in: pred/target [7, P, F] f32 (param-major, host-transposed).
    DRAM out: out [P, 1] f32, partial sum of (ratio - iou) over the shard.
    """
    nc = bass.Bass(trn_type="TRN2")
    pred_d = nc.dram_tensor("pred", [7, P, F], F32, kind="ExternalInput")
    targ_d = nc.dram_tensor("target", [7, P, F], F32, kind="ExternalInput")
    out_d = nc.dram_tensor("out", [P, 1], F32, kind="ExternalOutput")

    F12 = 12 * F
    F13 = 13 * F + 1

    import contextlib

    with tile.TileContext(nc) as tc, contextlib.ExitStack() as ctx:
        pool = ctx.enter_context(tc.tile_pool(name="main", bufs=1))
        V = nc.vector
        A = nc.scalar
        G = nc.gpsimd

        def tS(tag, w=1):
            return pool.tile([P, w * F], F32, tag=tag, name=tag)

        def tH(tag, w=1):
            return pool.tile([P, w * F], F16, tag=tag, name=tag)

        # ---- load inputs ----
        ins = {}
        bigs = {}
        for name, dram in (("p", pred_d), ("t", targ_d)):
            bigs[name] = (pool.tile([P, 7 * F], F32, tag=f"in_{name}",
                                    name=f"in_{name}"), dram)
        # angles first (trig heads the critical path), pred/targ interleaved
        for i in (6, 0, 1, 3, 4, 2, 5):
            for name in ("p", "t"):
                big, dram = bigs[name]
                nc.sync.dma_start(big[:, i * F:(i + 1) * F], dram[i])
                ins[f"{name}{i}"] = big[:, i * F:(i + 1) * F]
        x1, y1, z1 = ins["p0"], ins["p1"], ins["p2"]
        w1, h1, l1, ang1 = ins["p3"], ins["p4"], ins["p5"], ins["p6"]
        x2, y2, z2 = ins["t0"], ins["t1"], ins["t2"]
        w2, h2, l2, ang2 = ins["t3"], ins["t4"], ins["t5"], ins["t6"]

        # ---- center deltas (f32 on Pool, reused by ctd tail) ----
        dxc, dyc, dzc = tS("dxc"), tS("dyc"), tS("dzc")
        G.tensor_sub(dxc[:, :], x2[:, :], x1[:, :])
        G.tensor_sub(dyc[:, :], y2[:, :], y1[:, :])
        G.tensor_sub(dzc[:, :], z2[:, :], z1[:, :])
        dxch, dych = tH("dxch"), tH("dych")
        A.copy(dxch[:, :], dxc[:, :])
        A.copy(dych[:, :], dyc[:, :])

        # ---- trig: range reduce (angle1 on DVE, angle2 on Pool) + Act Sin
        trigf = {}
        for nm, at in (("1", ang1), ("2", ang2)):
            m = tS(f"trm{nm}")
            ar = tS(f"tra{nm}")
            sh = tS(f"trs{nm}")
            V.tensor_scalar(m[:, :], at[:, :], PI, None, Alu.is_ge)
            V.scalar_tensor_tensor(ar[:, :], m[:, :], -2 * PI, at[:, :],
                                   Alu.mult, Alu.add)
            V.tensor_scalar(m[:, :], ar[:, :], -PI, None, Alu.is_lt)
            V.scalar_tensor_tensor(ar[:, :], m[:, :], 2 * PI, ar[:, :],
                                   Alu.mult, Alu.add)
            V.tensor_scalar(m[:, :], ar[:, :], PI / 2, None, Alu.is_ge)
            V.scalar_tensor_tensor(sh[:, :], m[:, :], -2 * PI, ar[:, :],
                                   Alu.mult, Alu.add)
            V.tensor_scalar(sh[:, :], sh[:, :], PI / 2, None, Alu.add)
            trigf[f"ar{nm}"] = ar
            trigf[f"sh{nm}"] = sh
            # fp16 Sins first: they head the geometry critical path
            sh_ = tH(f"sin{nm}h")
            ch_ = tH(f"cos{nm}h")
            A.activation(sh_[:, :], ar[:, :], Act.Sin)
            A.activation(ch_[:, :], sh[:, :], Act.Sin)
            trigf[f"s{nm}h"] = sh_
            trigf[f"c{nm}h"] = ch_
        for nm in ("1", "2"):
            s_ = tS(f"sin{nm}")
            c_ = tS(f"cos{nm}")
            A.activation(s_[:, :], trigf[f"ar{nm}"][:, :], Act.Sin)
            A.activation(c_[:, :], trigf[f"sh{nm}"][:, :], Act.Sin)
            trigf[f"s{nm}"] = s_
            trigf[f"c{nm}"] = c_
        s1f, c1f = trigf["s1"], trigf["c1"]
        s2f, c2f = trigf["s2"], trigf["c2"]
        s1h, c1h = trigf["s1h"], trigf["c1h"]
        s2h, c2h = trigf["s2h"], trigf["c2h"]

        # ---- halfdims: fp16 for geometry, f32 (hl/hd) for the tail ----
        ah, bh = tH("ah"), tH("bh")
        hw2h, hh2h = tH("hw2h"), tH("hh2h")
        A.mul(ah[:, :], w1[:, :], 0.5)
        A.mul(bh[:, :], h1[:, :], 0.5)
        A.mul(hw2h[:, :], w2[:, :], 0.5)
        A.mul(hh2h[:, :], h2[:, :], 0.5)
        hl1, hl2 = tS("hl1"), tS("hl2")
        A.mul(hl1[:, :], l1[:, :], 0.5)
        A.mul(hl2[:, :], l2[:, :], 0.5)
        af, bf = tS("af"), tS("bf")
        A.mul(af[:, :], w1[:, :], 0.5)
        A.mul(bf[:, :], h1[:, :], 0.5)
        hw2f, hh2f = tS("hw2f"), tS("hh2f")
        A.mul(hw2f[:, :], w2[:, :], 0.5)
        A.mul(hh2f[:, :], h2[:, :], 0.5)
        # squared inflated half-dims: the in-box tests compare x^2 < lim^2
        infl = 1.0 + 2.0 * TOL
        iah, ibh = tH("iah"), tH("ibh")
        A.activation(iah[:, :], w1[:, :], Act.Square, scale=0.5 * infl)
        A.activation(ibh[:, :], h1[:, :], Act.Square, scale=0.5 * infl)
        ia2h, ib2h = tH("ia2h"), tH("ib2h")
        A.activation(ia2h[:, :], w2[:, :], Act.Square, scale=0.5 * infl)
        A.activation(ib2h[:, :], h2[:, :], Act.Square, scale=0.5 * infl)

        # ---- o = R(-a1)(c2-c1), pivot p = o/2 (fp16); runs on angle-1 trig
        # alone so it overlaps the angle-2 Sins still in the Act queue ----
        q1h, q2h = tH("q1h"), tH("q2h")
        oxh, oyh = tH("oxh"), tH("oyh")
        V.tensor_mul(q1h[:, :], dxch[:, :], c1h[:, :])
        V.tensor_mul(q2h[:, :], dych[:, :], s1h[:, :])
        V.tensor_add(oxh[:, :], q1h[:, :], q2h[:, :])
        V.tensor_mul(q1h[:, :], dxch[:, :], s1h[:, :])
        V.tensor_mul(q2h[:, :], dych[:, :], c1h[:, :])
        V.tensor_sub(oyh[:, :], q2h[:, :], q1h[:, :])

        # ---- delta trig (fp16) ----
        cth, sth = tH("cth"), tH("sth")
        V.tensor_mul(q1h[:, :], c1h[:, :], c2h[:, :])
        V.tensor_mul(q2h[:, :], s1h[:, :], s2h[:, :])
        V.tensor_add(cth[:, :], q1h[:, :], q2h[:, :])
        V.tensor_mul(q1h[:, :], s2h[:, :], c1h[:, :])
        V.tensor_mul(q2h[:, :], c2h[:, :], s1h[:, :])
        V.tensor_sub(sth[:, :], q1h[:, :], q2h[:, :])
        pxh, pyh = tH("pxh"), tH("pyh")
        A.mul(pxh[:, :], oxh[:, :], 0.5)
        A.mul(pyh[:, :], oyh[:, :], 0.5)

        # ---- U, V axis vectors of box2 in frame1 (fp16, packed [4F]) ----
        UV = tH("UV", 4)   # [Ux | Vx | Uy | Vy]
        Uxv, Vxv = UV[:, 0:F], UV[:, F:2 * F]
        Uyv, Vyv = UV[:, 2 * F:3 * F], UV[:, 3 * F:4 * F]
        V.tensor_mul(Uxv, hw2h[:, :], cth[:, :])
        V.scalar_tensor_tensor(Vxv, hh2h[:, :], -1.0, sth[:, :],
                               Alu.mult, Alu.mult)
        V.tensor_mul(Uyv, hw2h[:, :], sth[:, :])
        V.tensor_mul(Vyv, hh2h[:, :], cth[:, :])

        # ---- slot tiles (slot-major, 12 slots; cluster j = 3j..3j+2) ----
        VXs = tH("VXs", 12)
        VYs = tH("VYs", 12)
        MMs = tH("MMs", 12)

        def sl1(t, s):
            return _ap(t, s * F, [[1, F]])

        def slots(t, s0):
            return _ap(t, s0 * F, [[3 * F, 4], [1, F]])

        def bc4(t):
            return _ap(t, 0, [[0, 4], [1, F]])

        # box2 corners -> C2 slots (3j+1)
        tx1, tx2 = tH("tx1"), tH("tx2")
        sgn = [(1, 1), (-1, 1), (-1, -1), (1, -1)]
        for T_, o_, U0, V0 in ((VXs, oxh, Uxv, Vxv), (VYs, oyh, Uyv, Vyv)):
            V.tensor_add(tx1[:, :], o_[:, :], U0)
            V.tensor_sub(tx2[:, :], o_[:, :], U0)
            for j, (su, sv_) in enumerate(sgn):
                src = tx1 if su > 0 else tx2
                dst = sl1(T_, 3 * j + 1)
                if sv_ > 0:
                    V.tensor_add(dst, src[:, :], V0)
                else:
                    V.tensor_sub(dst, src[:, :], V0)

        # box1 corners (+-a, +-b) -> C1 slots (3j); x: [a,-a,-a,a] y: [b,b,-b,-b]
        nah, nbh = tH("nah"), tH("nbh")
        A.mul(nah[:, :], ah[:, :], -1.0)
        A.mul(nbh[:, :], bh[:, :], -1.0)
        V.tensor_copy(_ap(VXs, 0, [[9 * F, 2], [1, F]]), bc_pair(ah))
        V.tensor_copy(_ap(VXs, 3 * F, [[3 * F, 2], [1, F]]), bc_pair(nah))
        V.tensor_copy(_ap(VYs, 0, [[3 * F, 2], [1, F]]), bc_pair(bh))
        V.tensor_copy(_ap(VYs, 6 * F, [[3 * F, 2], [1, F]]), bc_pair(nbh))

        def r4(t):
            return _ap(t, 0, [[F, 4], [1, F]])

        # ---- Do (= +-2U,+-2V) and rD (= -+0.5/U, clamped) fp16 [4F] ----
        DoX, DoY = tH("DoX", 4), tH("DoY", 4)
        A.mul(DoX[:, :2 * F], UV[:, 0:2 * F], -2.0)
        A.mul(DoX[:, 2 * F:], UV[:, 0:2 * F], 2.0)
        A.mul(DoY[:, :2 * F], UV[:, 2 * F:], -2.0)
        A.mul(DoY[:, 2 * F:], UV[:, 2 * F:], 2.0)
        recX, recY = tS("recX", 2), tS("recY", 2)
        rDX, rDY = tH("rDX", 4), tH("rDY", 4)
        V.reciprocal(recY[:, :], UV[:, 2 * F:])
        V.tensor_scalar(recY[:, :], recY[:, :], RCLAMP, -RCLAMP,
                        Alu.min, Alu.max)
        V.tensor_scalar(rDY[:, :2 * F], recY[:, :], -0.5, None, Alu.mult)
        V.tensor_scalar(rDY[:, 2 * F:], recY[:, :], 0.5, None, Alu.mult)
        V.reciprocal(recX[:, :], UV[:, 0:2 * F])
        V.tensor_scalar(recX[:, :], recX[:, :], RCLAMP, -RCLAMP,
                        Alu.min, Alu.max)
        V.tensor_scalar(rDX[:, :2 * F], recX[:, :], -0.5, None, Alu.mult)
        V.tensor_scalar(rDX[:, 2 * F:], recX[:, :], 0.5, None, Alu.mult)

        # ---- edge x phantom-edge intersections -> IP slots (3j+2) ----
        levh, levv = tH("levh", 2), tH("levv", 2)
        A.copy(levh[:, :F], bh[:, :])
        A.copy(levh[:, F:], nbh[:, :])
        A.copy(levv[:, :F], nah[:, :])
        A.copy(levv[:, F:], ah[:, :])
        halfb = pool.tile([P, 1], F32, tag="halfb", name="halfb")
        G.memset(halfb[:, :], 0.5)
        sj8 = tH("sj8", 8)
        cc8 = tH("cc8", 8)
        abph = tH("abph", 8)
        abcc = tH("abcc", 8)
        mk8 = tH("mk8", 8)
        tm4 = tH("tm4", 4)
        xi, eta = tH("xi", 4), tH("eta", 4)
        o2x, o2y = tH("o2x"), tH("o2y")
        mac, mbs = tH("mac"), tH("mbs")

        def r8(t):
            return _ap(t, 0, [[4 * F, 2], [F, 4], [1, F]])

        def emit_m12_part(first):
            # m12 prep (slot-independent): fills the DVE stall while the
            # Act engine runs the two 8F Abs ops of this iteration
            if first:
                V.tensor_mul(q1h[:, :], oxh[:, :], cth[:, :])
                V.tensor_mul(q2h[:, :], oyh[:, :], sth[:, :])
                V.tensor_add(o2x[:, :], q1h[:, :], q2h[:, :])
                V.tensor_mul(q1h[:, :], oyh[:, :], cth[:, :])
                V.tensor_mul(q2h[:, :], oxh[:, :], sth[:, :])
                V.tensor_sub(o2y[:, :], q1h[:, :], q2h[:, :])
            else:
                V.tensor_mul(mac[:, :], ah[:, :], cth[:, :])
                V.tensor_mul(mbs[:, :], bh[:, :], sth[:, :])
                V.tensor_add(xi[:, 0:F], mac[:, :], mbs[:, :])      # A
                V.tensor_sub(xi[:, F:2 * F], mbs[:, :], mac[:, :])  # B
                V.tensor_scalar(xi[:, 2 * F:3 * F], xi[:, 0:F], -1.0, None,
                                Alu.mult)
                V.tensor_scalar(xi[:, 3 * F:], xi[:, F:2 * F], -1.0, None,
                                Alu.mult)
                V.tensor_mul(mac[:, :], bh[:, :], cth[:, :])
                V.tensor_mul(mbs[:, :], ah[:, :], sth[:, :])
                V.tensor_sub(eta[:, 0:F], mac[:, :], mbs[:, :])     # C
                V.tensor_add(eta[:, F:2 * F], mac[:, :], mbs[:, :])  # D
                V.tensor_scalar(eta[:, 2 * F:3 * F], eta[:, 0:F], -1.0, None,
                                Alu.mult)
                V.tensor_scalar(eta[:, 3 * F:], eta[:, F:2 * F], -1.0, None,
                                Alu.mult)

        for p_ in range(2):
            horiz = p_ == 0
            lev2 = levh if horiz else levv
            Qc = _ap(VYs if horiz else VXs, F, [[0, 2], [3 * F, 4], [1, F]])
            Qo = _ap(VXs if horiz else VYs, F, [[0, 2], [3 * F, 4], [1, F]])
            rD = _ap(rDY if horiz else rDX, 0, [[0, 2], [F, 4], [1, F]])
            Do = _ap(DoX if horiz else DoY, 0, [[0, 2], [F, 4], [1, F]])
            lev_b = _ap(lev2, 0, [[F, 2], [0, 4], [1, F]])
            lim_b = _ap(ah if horiz else bh, 0, [[0, 2], [0, 4], [1, F]])
            V.tensor_tensor(r8(sj8), lev_b, Qc, Alu.subtract)
            V.tensor_tensor(r8(sj8), r8(sj8), rD, Alu.mult)
            V.tensor_tensor(r8(cc8), r8(sj8), Do, Alu.mult)
            V.tensor_tensor(r8(cc8), r8(cc8), Qo, Alu.add)
            # ph = 1[-1 < sj8 < 0] = 1[|sj8 + 0.5| < 0.5]
            A.activation(r8(abph), r8(sj8), Act.Abs, bias=halfb[:, :])
            A.activation(r8(abcc), r8(cc8), Act.Abs)
            emit_m12_part(first=horiz)
            V.tensor_scalar(r8(abph), r8(abph), 0.5, None, Alu.is_lt)
            V.tensor_tensor(r8(abcc), r8(abcc), lim_b, Alu.is_lt)
            V.tensor_tensor(r8(mk8), r8(abph), r8(abcc), Alu.mult)
            # extraction for both edges (e_=0,1) at once via stride-2 views
            j0 = 0 if horiz else 1
            vs2 = _ap(VXs if horiz else VYs, (3 * j0 + 2) * F,
                      [[6 * F, 2], [1, F]])
            os2 = _ap(VYs if horiz else VXs, (3 * j0 + 2) * F,
                      [[6 * F, 2], [1, F]])
            mm2 = _ap(MMs, (3 * j0 + 2) * F, [[6 * F, 2], [1, F]])
            A.copy(vs2, _ap(cc8, 3 * F, [[4 * F, 2], [1, F]]))
            for k in (2, 1, 0):
                V.copy_predicated(
                    vs2,
                    _ap(mk8, k * F, [[4 * F, 2], [1, F]]).bitcast(I16),
                    _ap(cc8, k * F, [[4 * F, 2], [1, F]]))
            A.copy(os2, lev2[:, :])
            V.tensor_tensor(_ap(tm4, 0, [[2 * F, 2], [F, 2], [1, F]]),
                            _ap(mk8, 0, [[4 * F, 2], [F, 2], [1, F]]),
                            _ap(mk8, 2 * F, [[4 * F, 2], [F, 2], [1, F]]),
                            Alu.max)
            V.tensor_tensor(mm2, _ap(tm4, 0, [[2 * F, 2], [1, F]]),
                            _ap(tm4, F, [[2 * F, 2], [1, F]]), Alu.max)

        # ---- m21: c2 corners inside box1 -> MM slots 3j+1 ----
        # (squares and mask combines on Pool; only the compares stay on DVE)
        t4a, t4b = tH("t4a", 4), tH("t4b", 4)
        G.tensor_tensor(r4(t4a), slots(VXs, 1), slots(VXs, 1), Alu.mult)
        G.tensor_tensor(r4(t4b), slots(VYs, 1), slots(VYs, 1), Alu.mult)

        # ---- m12: c1 corners inside box2 -> MM slots 3j ----
        # corner j of box1 in frame2: xi_j = sx*a*ct + sy*b*st - o2x,
        # eta_j = sy*b*ct - sx*a*st - o2y  ->  xi = [A,B,-A,-B] - o2x,
        # eta = [C,D,-C,-D] - o2y with A=act+bst, B=bst-act, C=bct-ast,
        # D=bct+ast (all per-box scalars; assembled inside the 8F loop).
        V.tensor_tensor(r4(xi), r4(xi), bc4(o2x), Alu.subtract)
        V.tensor_tensor(r4(eta), r4(eta), bc4(o2y), Alu.subtract)
        G.tensor_tensor(r4(xi), r4(xi), r4(xi), Alu.mult)
        G.tensor_tensor(r4(eta), r4(eta), r4(eta), Alu.mult)
        V.tensor_tensor(r4(t4a), r4(t4a), bc4(iah), Alu.is_lt)
        V.tensor_tensor(r4(t4b), r4(t4b), bc4(ibh), Alu.is_lt)
        V.tensor_tensor(r4(xi), r4(xi), bc4(ia2h), Alu.is_lt)
        V.tensor_tensor(r4(eta), r4(eta), bc4(ib2h), Alu.is_lt)
        G.tensor_tensor(slots(MMs, 1), r4(t4a), r4(t4b), Alu.mult)
        G.tensor_tensor(slots(MMs, 0), r4(xi), r4(eta), Alu.mult)

        # ---- center on pivot, then mask ----
        def bc12(t):
            return _ap(t, 0, [[0, 12], [1, F]])

        V.tensor_tensor(VXs[:, :], VXs[:, :], bc12(pxh), Alu.subtract)
        V.tensor_tensor(VYs[:, :], VYs[:, :], bc12(pyh), Alu.subtract)
        V.tensor_mul(VXs[:, :], VXs[:, :], MMs[:, :])
        V.tensor_mul(VYs[:, :], VYs[:, :], MMs[:, :])

        # ---- local sort: one compare of slots 3j+1 / 3j+2 per cluster ----
        crA, crB = tH("crA", 4), tH("crB", 4)
        bkx, bky = tH("bkx", 4), tH("bky", 4)
        Ax_, Bx_ = slots(VXs, 1), slots(VXs, 2)
        Ay_, By_ = slots(VYs, 1), slots(VYs, 2)
        V.tensor_tensor(r4(crA), Ax_, By_, Alu.mult)
        V.tensor_tensor(r4(crB), Ay_, Bx_, Alu.mult)
        V.tensor_sub(crA[:, :], crA[:, :], crB[:, :])
        V.tensor_scalar(crA[:, :], crA[:, :], 0.0, None, Alu.is_lt)
        mwi = r4(crA).bitcast(I16)
        A.copy(r4(bkx), Ax_)
        A.copy(r4(bky), Ay_)
        V.copy_predicated(Ax_, mwi, Bx_)
        V.copy_predicated(Bx_, mwi, r4(bkx))
        V.copy_predicated(Ay_, mwi, By_)
        V.copy_predicated(By_, mwi, r4(bky))

        # ---- transform to box-major 13-slot layout (col0 + col 13f are 0) --
        TWXb = pool.tile([P, F13], F16, tag="TWXb", name="TWXb")
        TWYb = pool.tile([P, F13], F16, tag="TWYb", name="TWYb")
        TMpb = pool.tile([P, F13], F16, tag="TMpb", name="TMpb")
        for t_ in (TWXb, TWYb, TMpb):
            G.memset(_ap(t_, 0, [[13, F + 1]]), 0.0)

        def bm(t):
            return _ap(t, 1, [[13, F], [1, 12]])

        def bmL(t):
            return _ap(t, 0, [[13, F], [1, 12]])

        def r12T(t):
            return _ap(t, 0, [[1, F], [F, 12]])

        V.tensor_copy(bm(TWXb), r12T(VXs))
        V.tensor_copy(bm(TWYb), r12T(VYs))
        V.tensor_scalar(bm(TMpb), r12T(MMs), -1.0, 1.0, Alu.mult, Alu.add)

        # ---- last-valid scans (fwd x on DVE; fwd y + both rev on Pool) ----
        LX = pool.tile([P, F13], F16, tag="sj8", name="LX")
        LY = pool.tile([P, F13], F16, tag="cc8", name="LY")
        RLX = pool.tile([P, F13], F16, tag="abph", name="RLX")
        RLY = pool.tile([P, F13], F16, tag="abcc", name="RLY")

        def rev(t):
            return _ap(t, F13 - 1, [[-1, F13]])

        V.tensor_tensor_scan(LX[:, :], TMpb[:, :], TWXb[:, :], 0.0,
                             Alu.mult, Alu.add)
        V.tensor_tensor_scan(LY[:, :], TMpb[:, :], TWYb[:, :], 0.0,
                             Alu.mult, Alu.add)

        # ---- shoelace: C12 = Lprev_x*W_y - Lprev_y*W_x; Pool tree-sums the
        # slot products while the DVE runs the two reverse scans ----
        C12 = pool.tile([P, F13], F16, tag="mk8", name="C12")
        SC2 = pool.tile([P, F13], F16, tag="VXs", name="SC2")
        V.tensor_tensor(bm(C12), bmL(LX), bm(TWYb), Alu.mult)
        V.tensor_tensor(bm(SC2), bmL(LY), bm(TWXb), Alu.mult)
        V.tensor_tensor(bm(C12), bm(C12), bm(SC2), Alu.subtract)
        L1 = tH("L1", 6)
        L2 = tH("L2", 3)
        G.tensor_tensor(_ap(L1, 0, [[6, F], [1, 6]]),
                        _ap(C12, 1, [[13, F], [1, 6]]),
                        _ap(C12, 7, [[13, F], [1, 6]]), Alu.add)
        AREA2h = tH("AREA2h")
        q1w, q2w = tH("q1w"), tH("q2w")
        V.tensor_tensor_scan(rev(RLX), rev(TMpb), rev(TWXb), 0.0,
                             Alu.mult, Alu.add)
        V.tensor_tensor_scan(rev(RLY), rev(TMpb), rev(TWYb), 0.0,
                             Alu.mult, Alu.add)
        V.tensor_tensor(_ap(L2, 0, [[3, F], [1, 3]]),
                        _ap(L1, 0, [[6, F], [1, 3]]),
                        _ap(L1, 3, [[6, F], [1, 3]]), Alu.add)
        V.tensor_tensor(AREA2h[:, :], _ap(L2, 0, [[3, F]]),
                        _ap(L2, 1, [[3, F]]), Alu.add)
        V.tensor_tensor(AREA2h[:, :], AREA2h[:, :], _ap(L2, 2, [[3, F]]),
                        Alu.add)
        # wrap: cross(last_valid, first_valid)
        V.tensor_tensor(q1w[:, :], _ap(LX, 12, [[13, F]]),
                        _ap(RLY, 1, [[13, F]]), Alu.mult)
        V.tensor_tensor(q2w[:, :], _ap(LY, 12, [[13, F]]),
                        _ap(RLX, 1, [[13, F]]), Alu.mult)
        V.tensor_sub(q1w[:, :], q1w[:, :], q2w[:, :])
        V.tensor_add(AREA2h[:, :], AREA2h[:, :], q1w[:, :])
        # ---- independent f32 tail: z overlap, vols, ctd/cnd/did, ratio ----
        # (emitted here so Pool/Act chew through it while the DVE runs the
        # polygon machinery; only the iou/loss combine stays at the end)
        zx1 = tS("zx1")
        zn1 = tS("zn1")
        zx2 = tS("zx2")
        zn2 = tS("zn2")
        G.tensor_add(zx1[:, :], z1[:, :], hl1[:, :])
        G.tensor_sub(zn1[:, :], z1[:, :], hl1[:, :])
        G.tensor_add(zx2[:, :], z2[:, :], hl2[:, :])
        G.tensor_sub(zn2[:, :], z2[:, :], hl2[:, :])
        vol1 = tS("vol1")
        vol2 = tS("vol2")
        G.tensor_mul(vol1[:, :], w1[:, :], h1[:, :])
        G.tensor_mul(vol1[:, :], vol1[:, :], l1[:, :])
        G.tensor_mul(vol2[:, :], w2[:, :], h2[:, :])
        G.tensor_mul(vol2[:, :], vol2[:, :], l2[:, :])
        G.tensor_add(vol1[:, :], vol1[:, :], vol2[:, :])
        sq1, sq2 = tS("sq1"), tS("sq2")
        ctd = tS("ctd")
        A.activation(sq1[:, :], dxc[:, :], Act.Square)
        A.activation(sq2[:, :], dyc[:, :], Act.Square)
        G.tensor_add(ctd[:, :], sq1[:, :], sq2[:, :])
        A.activation(sq1[:, :], dzc[:, :], Act.Square)
        G.tensor_add(ctd[:, :], ctd[:, :], sq1[:, :])
        pr = {}
        for nm, (d_, t_) in (("p11", (af, c1f)), ("p12", (hl1, s1f)),
                             ("p13", (af, s1f)), ("p14", (hl1, c1f)),
                             ("p21", (hw2f, c2f)), ("p22", (hl2, s2f)),
                             ("p23", (hw2f, s2f)), ("p24", (hl2, c2f))):
            tl = tS(f"pr_{nm}")
            G.tensor_mul(tl[:, :], d_[:, :], t_[:, :])
            pr[nm] = tl
        quad = tS("quad")
        gg = tS("gg")
        G.tensor_sub(gg[:, :], pr["p11"][:, :], pr["p21"][:, :])
        A.activation(quad[:, :], gg[:, :], Act.Square)
        for x_, y_ in (("p12", "p22"), ("p23", "p13"), ("p14", "p24")):
            G.tensor_sub(gg[:, :], pr[x_][:, :], pr[y_][:, :])
            A.activation(sq1[:, :], gg[:, :], Act.Square)
            G.tensor_add(quad[:, :], quad[:, :], sq1[:, :])
        G.tensor_sub(gg[:, :], bf[:, :], hh2f[:, :])
        A.activation(sq1[:, :], gg[:, :], Act.Square)
        G.tensor_add(quad[:, :], quad[:, :], sq1[:, :])
        did = tS("did")
        A.activation(sq1[:, :], w2[:, :], Act.Square)
        A.activation(sq2[:, :], h2[:, :], Act.Square)
        G.tensor_add(did[:, :], sq1[:, :], sq2[:, :])
        A.activation(sq1[:, :], l2[:, :], Act.Square)
        G.tensor_add(did[:, :], did[:, :], sq1[:, :])

        # V-side tail pieces: by now the Pool ctd/quad/did chain is long done
        V.tensor_tensor(zx1[:, :], zx1[:, :], zx2[:, :], Alu.min)
        V.tensor_max(zn1[:, :], zn1[:, :], zn2[:, :])
        V.tensor_sub(zx1[:, :], zx1[:, :], zn1[:, :])
        V.tensor_scalar(zx1[:, :], zx1[:, :], 0.0, None, Alu.max)  # z overlap
        S_ = tS("S_")
        V.scalar_tensor_tensor(S_[:, :], ctd[:, :], 2.0, quad[:, :],
                               Alu.mult, Alu.add)
        den2 = tS("den2")
        V.scalar_tensor_tensor(den2[:, :], did[:, :], LOSS_EPS, S_[:, :],
                               Alu.add, Alu.add)
        rden2 = tS("rden2")
        V.reciprocal(rden2[:, :], den2[:, :])
        ratio = tS("ratio")
        V.tensor_mul(ratio[:, :], S_[:, :], rden2[:, :])

        AREA = tS("AREA")
        A.activation(AREA[:, :], AREA2h[:, :], Act.Abs, scale=0.5)
        inter3 = tS("inter3")
        V.tensor_mul(inter3[:, :], AREA[:, :], zx1[:, :])
        den = tS("den")
        V.tensor_sub(den[:, :], vol1[:, :], inter3[:, :])
        rden = tS("rden")
        V.reciprocal(rden[:, :], den[:, :])
        iou = tS("iou")
        V.tensor_mul(iou[:, :], inter3[:, :], rden[:, :])
        pl = tS("pl")
        partial = pool.tile([P, 1], F32, tag="partial", name="partial")
        V.scalar_tensor_tensor(pl[:, :], iou[:, :], -1.0, ratio[:, :],
                               Alu.mult, Alu.add, accum_out=partial[:, :])
        nc.sync.dma_start(out_d[:, :], partial[:, :])

    if legalize:
        _legalize_sync(nc)
    return nc


_NC_CACHE = {}


def _get_nc(F):
    if F not in _NC_CACHE:
        _NC_CACHE[F] = build_nc(F)
    return _NC_CACHE[F]


def kernel(pred: np.ndarray, target: np.ndarray) -> np.ndarray:
    N = pred.shape[0]
    per_core = N // NCORES
    F = per_core // P
    nc = _get_nc(F)
    in_maps = []
    for c in range(NCORES):
        sl = slice(c * per_core, (c + 1) * per_core)
        pm = np.ascontiguousarray(
            pred[sl].astype(np.float32).T.reshape(7, P, F))
        tm = np.ascontiguousarray(
            target[sl].astype(np.float32).T.reshape(7, P, F))
        in_maps.append({"pred": pm, "target": tm})
    res = run_bass_kernel_spmd(nc, in_maps, core_ids=list(range(NCORES)))
    total = 0.0
    for r in res.results:
        total += float(np.sum(r["out"].astype(np.float64)))
    return np.float32(1.0 + total / N)
